# revision 17
# baseline (speedup 1.0000x reference)
"""GQA attention kernel for 8 Trainium2 NeuronCores.

Sharding: batch x head-group. Core c handles batch b = c // 4 and head
group g = c % 4 (8 q heads 8g..8g+7, kv heads 2g, 2g+1). Each core
computes a partial output  attn_out_g[b] @ w_out[rows of g]  and the
host sums the 4 partials per batch.

Single-pass pipelined design: the kernel runs a slab-major loop (4 token
slabs of 512).  For slab s it computes attention for q-slab s (scores ->
exp -> PV, transposed layout, softmax over the partition dim with the
denominator from an appended ones-column in V).  The QKV projection of
slab s+1 and the out-projection of slab s-1 are chopped into small
matmul chunks and woven into the attention loop's emission order so the
PE array stays busy while the Scalar engine works through the exp
stream.  x is transposed on the host so activations stream in with
plain DMAs.  Softmax reciprocal is computed as exp(-log(den)) on the
Scalar engine (one table set for the whole kernel).
"""

import numpy as np
import ml_dtypes

B, T, D = 2, 2048, 2048
H, KVH, HD = 32, 8, 64
KVD = KVH * HD  # 512
NCORES = 8
SCALE = 1.0 / np.sqrt(HD)

DT = D // 128   # 16 d-tiles
NSLAB = 4
SLAB = 512

_CACHE = {}


def _build():
    import concourse.bass as bass
    import concourse.mybir as mybir
    import concourse.tile as tile
    from concourse import bacc

    f32 = mybir.dt.float32
    bf16 = mybir.dt.bfloat16
    AF = mybir.ActivationFunctionType
    OP = mybir.AluOpType

    nc = bacc.Bacc("TRN2", target_bir_lowering=False, debug=False)

    xbT = nc.dram_tensor("xbT", [D, T], bf16, kind="ExternalInput")
    wqk = nc.dram_tensor("wqk", [D, 640], bf16, kind="ExternalInput")
    wv = nc.dram_tensor("wv", [D, 128], bf16, kind="ExternalInput")
    wo = nc.dram_tensor("wo", [512, D], bf16, kind="ExternalInput")
    sinT = nc.dram_tensor("sinT", [128, T], bf16, kind="ExternalInput")
    cosT = nc.dram_tensor("cosT", [128, T], bf16, kind="ExternalInput")
    perm = nc.dram_tensor("perm", [128, 128], bf16, kind="ExternalInput")
    ident = nc.dram_tensor("ident", [64, 64], bf16, kind="ExternalInput")
    ident128 = nc.dram_tensor("ident128", [128, 128], bf16, kind="ExternalInput")
    masks = nc.dram_tensor("masks", [4 * 128, 1024], bf16, kind="ExternalInput")
    outp = nc.dram_tensor("outp", [T, D], bf16, kind="ExternalOutput")

    with tile.TileContext(nc) as tc:
        with (
            tc.tile_pool(name="const", bufs=1) as cpool,
            tc.tile_pool(name="resid", bufs=1) as rpool,
            tc.tile_pool(name="xp", bufs=2) as xpool,
            tc.tile_pool(name="pr", bufs=3) as pr,
            tc.tile_pool(name="pb", bufs=4) as pb,
            tc.tile_pool(name="pn", bufs=2) as pn,
            tc.tile_pool(name="pc", bufs=3) as pc,
            tc.tile_pool(name="ps_sc", bufs=2, space="PSUM") as ps_sc,
            tc.tile_pool(name="ps_pv", bufs=1, space="PSUM") as ps_pv,
            tc.tile_pool(name="ps_acc", bufs=1, space="PSUM") as ps_acc,
            tc.tile_pool(name="ps_aux", bufs=1, space="PSUM") as ps_aux,
        ):
            # ---- resident constants ----
            wqk_sb = [cpool.tile([128, 640], bf16, tag=f"wqk{i}", name=f"wqk{i}") for i in range(DT)]
            wv_sb = [cpool.tile([128, 128], bf16, tag=f"wv{i}", name=f"wv{i}") for i in range(DT)]
            wo_sb = [cpool.tile([128, D], bf16, tag=f"wo{i}", name=f"wo{i}") for i in range(4)]
            sin_sb = cpool.tile([128, T], bf16, tag="sin")
            cos_sb = cpool.tile([128, T], bf16, tag="cos")
            perm_sb = cpool.tile([128, 128], bf16, tag="perm")
            ident_sb = cpool.tile([64, 64], bf16, tag="ident")
            id128_sb = cpool.tile([128, 128], bf16, tag="id128")
            mask_sb = [cpool.tile([128, 2, 512], bf16, tag=f"mask{r}", name=f"mask{r}") for r in range(4)]
            ones_sb = cpool.tile([1, 64], bf16, tag="ones")

            # ---- persistent activations (per-slab tiles for precise deps) ----
            # qk[e][s]: e=0..3 q head-pairs (head e in parts 0:64, head e+4 in
            # 64:128), e=4: kT (kv head 2g in 0:64, 2g+1 in 64:128)
            qk = [[rpool.tile([128, SLAB], bf16, tag=f"qk{e}_{s}", name=f"qk{e}_{s}")
                   for s in range(NSLAB)] for e in range(5)]
            vnat = [rpool.tile([128, 130], bf16, tag=f"vn{k}", name=f"vn{k}") for k in range(16)]
            attnT = [[rpool.tile([128, SLAB], bf16, tag=f"at{j}_{s}", name=f"at{j}_{s}")
                      for s in range(NSLAB)] for j in range(4)]

            # Pin the one activation table set (ln+exp) up front so the
            # finalize pass doesn't thrash exp_and_others <-> natural_log.
            nc.scalar.add_instruction(mybir.InstLoadActFuncSet(
                name=nc.get_next_instruction_name(), act_func_set_id=6,
                ins=[], outs=[]))

            # ---- DMA issue: x slab first, then weights in need-order ----
            xbT3 = xbT.rearrange("(o p) t -> p o t", p=128)
            xts = [None] * NSLAB

            def issue_x(s):
                t = xpool.tile([128, DT, SLAB], bf16, tag="xT", name=f"xT{s}")
                nc.sync.dma_start(t[:], xbT3[:, :, s * SLAB:(s + 1) * SLAB])
                xts[s] = t

            wqk3 = wqk.rearrange("(o p) e -> p o e", p=128)
            wv3 = wv.rearrange("(o p) e -> p o e", p=128)
            wo3 = wo.rearrange("(o p) e -> p o e", p=128)
            x0 = xpool.tile([128, DT, SLAB], bf16, tag="xT", name="xT0")
            xts[0] = x0
            for d0 in range(0, DT, 4):
                nc.sync.dma_start(x0[:, d0:d0 + 4, :],
                                  xbT3[:, d0:d0 + 4, 0:SLAB])
                for i in range(d0, d0 + 4):
                    nc.sync.dma_start(wqk_sb[i][:], wqk3[:, i])
            nc.sync.dma_start(sin_sb[:], sinT[:])
            nc.sync.dma_start(cos_sb[:], cosT[:])
            nc.sync.dma_start(perm_sb[:], perm[:])
            nc.sync.dma_start(ident_sb[:], ident[:])
            nc.sync.dma_start(id128_sb[:], ident128[:])
            m4 = masks.rearrange("(r p) (h q) -> r p h q", p=128, h=2)
            for r in range(4):
                nc.sync.dma_start(mask_sb[r][:], m4[r])
            for i in range(DT):
                nc.sync.dma_start(wv_sb[i][:], wv3[:, i])
            nc.gpsimd.memset(ones_sb[:], 1.0)
            issue_x(1)
            for i in range(4):
                nc.sync.dma_start(wo_sb[i][:], wo3[:, i])

            # ================= phase A chunk machinery =================
            def finish_e(s, e, acc):
                ssl = slice(s * SLAB, (s + 1) * SLAB)
                raw = pr.tile([128, SLAB], bf16, tag="raw", name="raw")
                nc.vector.tensor_copy(raw[:], acc[:])
                if e == 5:
                    for i in range(4):
                        kt = 4 * s + i
                        vtp = ps_aux.tile([128, 128], bf16, tag="aux", name="vtp")
                        nc.tensor.transpose(
                            vtp[:], raw[:, i * 128:(i + 1) * 128], id128_sb[:])
                        nc.gpsimd.memset(vnat[kt][:], 1.0)
                        nc.vector.tensor_copy(vnat[kt][:, 0:64], vtp[:, 0:64])
                        nc.vector.tensor_copy(vnat[kt][:, 65:129], vtp[:, 64:128])
                    return
                rot = ps_aux.tile([128, SLAB], f32, tag="aux", name="rot")
                nc.tensor.matmul(rot[:], perm_sb[:], raw[:], start=True, stop=True)
                m2 = pr.tile([128, SLAB], bf16, tag="m2", name="m2")
                nc.vector.tensor_tensor(m2[:], raw[:], cos_sb[:, ssl], OP.mult)
                m1 = pr.tile([128, SLAB], bf16, tag="m1", name="m1")
                nc.vector.tensor_tensor(m1[:], rot[:], sin_sb[:, ssl], OP.mult)
                nc.vector.tensor_tensor(qk[e][s][:], m1[:], m2[:], OP.add)

            def a_chunks(s):
                chunks = []
                for e in range(6):
                    accbox = [None]
                    for c0 in range(0, DT, 2):
                        def mk(e=e, c0=c0, accbox=accbox, s=s):
                            def f():
                                if c0 == 0:
                                    accbox[0] = ps_acc.tile([128, SLAB], f32, tag="acc", name="acc")
                                acc = accbox[0]
                                wsrc = wqk_sb if e < 5 else wv_sb
                                ecol = e * 128 if e < 5 else 0
                                for d in (c0, c0 + 1):
                                    nc.tensor.matmul(
                                        acc[:], wsrc[d][:, ecol:ecol + 128],
                                        xts[s][:, d, :],
                                        start=(d == 0), stop=(d == DT - 1),
                                    )
                                if c0 == DT - 2:
                                    finish_e(s, e, acc)
                            return f
                        chunks.append(mk())
                return chunks

            # ================= phase C chunk machinery =================
            def c_chunks(s, pool=None, ptag="aux"):
                pool = pool or ps_aux
                chunks = []
                for it in range(4):
                    i = 4 * s + it
                    for ns in range(4):
                        def mk(i=i, it=it, ns=ns, s=s):
                            def f():
                                po = pool.tile([128, 512], f32, tag=ptag, name="po")
                                for j in range(4):
                                    nc.tensor.matmul(
                                        po[:],
                                        attnT[j][s][:, it * 128:(it + 1) * 128],
                                        wo_sb[j][:, ns * 512:(ns + 1) * 512],
                                        start=(j == 0), stop=(j == 3),
                                    )
                                ot = pc.tile([128, 512], bf16, tag="ot", name="ot")
                                nc.vector.tensor_copy(ot[:], po[:])
                                nc.sync.dma_start(
                                    outp[i * 128:(i + 1) * 128,
                                         ns * 512:(ns + 1) * 512], ot[:])
                            return f
                        chunks.append(mk())
                return chunks

            # ================= norm (stage-batched) =================
            # All 4 head-pair denominators of a stage share one ln+exp on a
            # [1, 4096] row, emitted at the stage boundary while the PE
            # drains background chunks -- the exp stream inside the
            # attention loop is never interrupted.
            def norm_part1(stg):
                lnr = pn.tile([1, 4096], f32, tag="lnr", name="lnr", bufs=1)
                nc.scalar.activation(lnr[:], stg[64:65, :], AF.Ln)
                rec = pn.tile([1, 4096], bf16, tag="rec", name="rec", bufs=1)
                nc.scalar.activation(rec[:], lnr[:], AF.Exp, scale=-1.0)
                return rec

            def norm_part2(j, s, stg, rec):
                c0 = j * 1024
                mv = ps_aux.tile([128, 512], f32, tag="aux", name="mv")
                nc.tensor.matmul(mv[64:128], ident_sb[:],
                                 stg[0:64, c0 + 512:c0 + 1024],
                                 start=True, stop=True)
                stg2 = pn.tile([128, 512], bf16, tag="stg2", name="stg2")
                nc.vector.tensor_copy(stg2[64:128], mv[64:128])
                bc = ps_aux.tile([128, 512], f32, tag="aux", name="bc")
                nc.tensor.matmul(bc[0:64], ones_sb[0:1, 0:64],
                                 rec[0:1, c0:c0 + 512], start=True, stop=True)
                nc.tensor.matmul(bc[64:128], ones_sb[0:1, 0:64],
                                 rec[0:1, c0 + 512:c0 + 1024],
                                 start=True, stop=True)
                nc.vector.tensor_tensor(
                    attnT[j][s][0:64, :], stg[0:64, c0:c0 + 512],
                    bc[0:64], OP.mult)
                nc.vector.tensor_tensor(
                    attnT[j][s][64:128, :], stg2[64:128], bc[64:128], OP.mult)

            # ================= main pipelined loop =================
            def mix(a, c):
                out = []
                ia = ic = 0
                while ia < len(a) or ic < len(c):
                    for _ in range(3):
                        if ia < len(a):
                            out.append(a[ia]); ia += 1
                    if ic < len(c):
                        out.append(c[ic]); ic += 1
                return out

            bg = []          # background PE chunks (A of s+1, C of s-1)

            # phase A for slab 0 runs densely up front
            for ch in a_chunks(0):
                ch()

            for s in range(NSLAB):
                if s < NSLAB - 1:
                    if s + 2 < NSLAB:
                        issue_x(s + 2)
                    newa = a_chunks(s + 1)
                else:
                    newa = []
                nkt = 4 * s + 4
                bg = mix(newa, bg)
                if s >= 1:
                    bg = mix(bg, c_chunks(s - 1))
                stg = pn.tile([65, 4096], bf16, tag="stg", name="stg", bufs=1)
                for j in range(4):
                    pvt = ps_pv.tile([128, 1024], f32, tag="pv", name="pv")
                    probs = {}
                    rag = {}
                    for kt in range(nkt + 1):
                        if kt < nkt:
                            # ragged causal cut: for diagonal k-tiles only
                            # q-columns >= 128*r can be unmasked
                            r = kt - 4 * s
                            q0 = 128 * r if r > 0 else 0
                            rag[kt] = q0
                            sc = ps_sc.tile([128, 2, 512], f32, tag="sc", name="sc")
                            for h, base in ((0, 0), (1, 64)):
                                nc.tensor.matmul(
                                    sc[:, h, q0:512],
                                    qk[4][kt // 4][base:base + 64,
                                                   (kt % 4) * 128:(kt % 4 + 1) * 128],
                                    qk[j][s][base:base + 64, q0:512],
                                    start=True, stop=True,
                                )
                            p = pb.tile([128, 2, 512], bf16, tag="probs", name="probs")
                            nc.scalar.activation(p[:, :, q0:512], sc[:, :, q0:512],
                                                 AF.Exp, scale=float(SCALE))
                            if r >= 0:
                                nc.vector.tensor_tensor(
                                    p[:, :, q0:512], p[:, :, q0:512],
                                    mask_sb[r][:, :, q0:512], OP.mult)
                            probs[kt] = p
                        if kt >= 1:
                            k0 = kt - 1
                            qq0 = rag[k0]
                            nc.tensor.matmul(
                                pvt[0:65, qq0:512], vnat[k0][:, 0:65],
                                probs[k0][:, 0, qq0:512],
                                start=(k0 == 0), stop=(k0 == nkt - 1),
                            )
                            nc.tensor.matmul(
                                pvt[0:65, 512 + qq0:1024], vnat[k0][:, 65:130],
                                probs[k0][:, 1, qq0:512],
                                start=(k0 == 0), stop=(k0 == nkt - 1),
                            )
                        if bg:
                            bg.pop(0)()
                    # free the pv banks quickly, keep denominator row
                    nc.vector.tensor_copy(
                        stg[:, j * 1024:(j + 1) * 1024], pvt[0:65, :])

                # stage boundary: ln+exp over all 4 denominators runs on the
                # Scalar engine while the PE drains background chunks, then
                # the normalizations complete.
                rec = norm_part1(stg)
                for ch in bg:
                    ch()
                bg = []
                for j in range(4):
                    norm_part2(j, s, stg, rec)

            # out-projection of the last slab (sc banks are free by now ->
            # double-buffered PSUM keeps the tail dense)
            for ch in c_chunks(NSLAB - 1, pool=ps_sc, ptag="sc"):
                ch()

    nc.finalize()
    return nc


def _host_inputs(x, sin, cos, w_qkv, w_out):
    bf = ml_dtypes.bfloat16
    sinT_np = np.concatenate([sin.T, sin.T], axis=0).astype(bf)  # [128, T]
    cosT_np = np.concatenate([cos.T, cos.T], axis=0).astype(bf)

    perm_np = np.zeros((128, 128), np.float32)
    for blk in range(2):
        for p in range(64):
            k = blk * 64 + ((p + 32) % 64)
            perm_np[k, blk * 64 + p] = -1.0 if p < 32 else 1.0
    perm_np = perm_np.astype(bf)
    ident_np = np.eye(64, dtype=np.float32).astype(bf)
    id128_np = np.eye(128, dtype=np.float32).astype(bf)

    mask_np = np.zeros((4, 128, 1024), np.float32)
    cix = np.arange(512)[None, :]
    pix = np.arange(128)[:, None]
    for r in range(4):
        m = (cix >= 128 * r + pix).astype(np.float32)
        mask_np[r, :, 0:512] = m
        mask_np[r, :, 512:1024] = m
    mask_np = mask_np.reshape(512, 1024).astype(bf)

    in_maps = []
    for c in range(NCORES):
        b, g = divmod(c, 4)
        cols = []
        for j in range(4):
            h1, h2 = 8 * g + j, 8 * g + 4 + j
            cols.append(w_qkv[:, 64 * h1:64 * h1 + 64])
            cols.append(w_qkv[:, 64 * h2:64 * h2 + 64])
        cols.append(w_qkv[:, D + 128 * g: D + 128 * g + 128])  # k heads 2g,2g+1
        wqk_np = np.concatenate(cols, axis=1).astype(bf)
        wv_np = w_qkv[:, D + KVD + 128 * g: D + KVD + 128 * g + 128].astype(bf)
        rows = []
        for j in range(4):
            h1, h2 = 8 * g + j, 8 * g + 4 + j
            rows.append(w_out[64 * h1:64 * h1 + 64, :])
            rows.append(w_out[64 * h2:64 * h2 + 64, :])
        wo_np = np.concatenate(rows, axis=0).astype(bf)
        in_maps.append({
            "xbT": np.ascontiguousarray(x[b].T).astype(bf),
            "wqk": wqk_np,
            "wv": wv_np,
            "wo": wo_np,
            "sinT": sinT_np,
            "cosT": cosT_np,
            "perm": perm_np,
            "ident": ident_np,
            "ident128": id128_np,
            "masks": mask_np,
        })
    return in_maps


def kernel(x, sin, cos, w_qkv, w_out, _trace=False):
    from concourse.bass_utils import run_bass_kernel_spmd

    if "nc" not in _CACHE:
        _CACHE["nc"] = _build()
    nc = _CACHE["nc"]

    in_maps = _host_inputs(
        np.asarray(x), np.asarray(sin), np.asarray(cos),
        np.asarray(w_qkv), np.asarray(w_out))
    res = run_bass_kernel_spmd(
        nc, in_maps, core_ids=list(range(NCORES)), trace=_trace)
    out = np.zeros((B, T, D), np.float32)
    for c in range(NCORES):
        b = c // 4
        out[b] += res.results[c]["outp"]
    if _trace:
        kernel.last_result = res
    return out
